# revision 20
# baseline (speedup 1.0000x reference)
"""DeepSet segment-reduce kernel for 8 Trainium2 NeuronCores (Bass/Tile).

Math (reference):
    h  = relu(x1 @ W1 + b1) @ W2 + b2          # [E, 128]
    S  = segment_sum(h, seg)                   # [B, 128]
    mean = S / max(counts, 1)
    out  = mean @ W3 + b3

Because segment-sum is linear, only r = relu(x1 @ W1 + b1) needs per-edge
work:  segsum(h) = segsum(r) @ W2 + counts x b2.  So the kernel:

  host: repack edges so every segment starts on a 128-edge block boundary
        (pad columns use x_pad with W1.T x_pad + b1 < 0, so relu kills them
        exactly), shard segment runs across 8 cores balanced by block count.
  core: stream xT [128, E_cap] tiles; hT = W1.T @ xT (PE, W1 stationary);
        relu+bias on ACT (PSUM->SBUF); per-128-block sums on DVE (3-D
        tensor_reduce);  ST[f, b] = block_sumsT @ A  via PE transposes +
        matmuls where A [J, B] is the per-core one-hot block->segment map
        (an input tensor, so the instruction stream is identical on all
        cores -> one SPMD program);
        AllReduce partial ST over the 8 cores;
        sums_hT = W2.T @ S + b2 x counts (rank-1 via k=1 matmul);
        meanT   = sums_hT * inv_counts (broadcast tensor passed as input);
        out     = meanT.T @ W3 + 1 x b3 (rank-1), DMA to [B, 128] output.

Self-contained: no reads of /root/problem/*; shapes derived from inputs.
"""

import math

import numpy as np

N_CORES = 8
BLOCK = 128          # segment alignment quantum (edges per block)
DMA_TILE = 4096      # xT columns per DMA (2 MiB)
PSUM_TILE = 1024     # columns per PSUM tile / ACT op (2 banks)
PAD_MARGIN = 8.0


def _plan_shards(edge_slices, E, B):
    es = np.asarray(edge_slices, dtype=np.int64)
    counts = (es[1:] - es[:-1]).astype(np.int64)        # [B]
    seg_blocks = (counts + BLOCK - 1) // BLOCK          # [B]
    total_blocks = int(seg_blocks.sum())

    # contiguous runs of segments per core, balanced by block count
    cum = np.cumsum(seg_blocks)
    bounds = [0]
    for c in range(1, N_CORES):
        bounds.append(int(np.searchsorted(cum, c * total_blocks / N_CORES)))
    bounds.append(B)

    core_blocks = []
    for c in range(N_CORES):
        core_blocks.append(int(seg_blocks[bounds[c]:bounds[c + 1]].sum()))
    j_max = max(core_blocks)
    e_cap = int(math.ceil(j_max * BLOCK / DMA_TILE) * DMA_TILE)
    return es, counts, seg_blocks, bounds, e_cap


def _solve_xpad(W1, b1):
    # x_pad with W1.T x_pad + b1 = -PAD_MARGIN elementwise => relu output 0
    rhs = -(b1.astype(np.float64) + PAD_MARGIN)
    x_pad = np.linalg.solve(W1.astype(np.float64).T, rhs)
    chk = W1.astype(np.float64).T @ x_pad + b1.astype(np.float64)
    assert chk.max() < -1.0, "x_pad margin too small"
    return x_pad.astype(np.float32)


def _build_core_inputs(x1, es, counts, seg_blocks, bounds, e_cap, x_pad, B):
    J = e_cap // BLOCK
    J_pad = ((J + 127) // 128) * 128
    xTs, segids = [], []
    for c in range(N_CORES):
        xT = np.empty((128, e_cap), dtype=np.float32)
        segid = np.full((1, J_pad), -1.0, dtype=np.float32)
        pos = 0
        for b in range(bounds[c], bounds[c + 1]):
            cnt = int(counts[b])
            if cnt == 0:
                continue
            xT[:, pos:pos + cnt] = x1[es[b]:es[b + 1], :].T
            nb = int(seg_blocks[b])
            pad = nb * BLOCK - cnt
            if pad:
                xT[:, pos + cnt:pos + nb * BLOCK] = x_pad[:, None]
            segid[0, pos // BLOCK: pos // BLOCK + nb] = float(b)
            pos += nb * BLOCK
        if pos < e_cap:
            xT[:, pos:] = x_pad[:, None]
        xTs.append(xT)
        segids.append(segid)
    return xTs, segids, J


def _build_bass(e_cap, J, B):
    n_chunks = (J + 127) // 128
    J_pad = n_chunks * 128
    import concourse.bacc as bacc
    import concourse.mybir as mybir
    import concourse.tile as tile

    f32 = mybir.dt.float32
    Relu = mybir.ActivationFunctionType.Relu

    nc = bacc.Bacc(trn_type="TRN2", num_devices=N_CORES)

    f32r = mybir.dt.float32r
    xT_d = nc.dram_tensor("xT", [128, e_cap], f32r, kind="ExternalInput")
    seg_d = nc.dram_tensor("segid", [1, J_pad], f32, kind="ExternalInput")
    W1_d = nc.dram_tensor("W1", [128, 128], f32r, kind="ExternalInput")
    b1_d = nc.dram_tensor("b1c", [128, 1], f32, kind="ExternalInput")
    W2_d = nc.dram_tensor("W2", [128, 128], f32r, kind="ExternalInput")
    b2_d = nc.dram_tensor("b2r", [1, 128], f32r, kind="ExternalInput")
    W3_d = nc.dram_tensor("W3", [128, 128], f32, kind="ExternalInput")
    b3_d = nc.dram_tensor("b3r", [1, 128], f32, kind="ExternalInput")
    cnt_d = nc.dram_tensor("counts_row", [1, B], f32r, kind="ExternalInput")
    inv_d = nc.dram_tensor("inv_bcast", [128, B], f32, kind="ExternalInput")
    ones_d = nc.dram_tensor("ones_row", [1, 128], f32, kind="ExternalInput")
    ident_d = nc.dram_tensor("ident", [128, 128], f32, kind="ExternalInput")
    out_d = nc.dram_tensor("out", [B, 128], f32, kind="ExternalOutput")

    n_dma = e_cap // DMA_TILE
    n_half = DMA_TILE // PSUM_TILE
    blk_per_ps = PSUM_TILE // BLOCK

    with tile.TileContext(nc) as tc, tc.tile_pool(name="persist", bufs=1) as pp:
        # persistent tiles (distinct tags -> own slot each)
        w1_sb = pp.tile([128, 128], f32r, name="w1_sb")
        b1_sb = pp.tile([128, 1], f32, name="b1_sb")
        ident_sb = pp.tile([128, 128], f32, name="ident_sb")
        bs_sb = pp.tile([128, J], f32, name="bs_sb")
        bsT_sb = pp.tile([128, n_chunks * 128], f32, name="bsT_sb")
        nc.sync.dma_start(w1_sb[:], W1_d[:])
        nc.sync.dma_start(b1_sb[:], b1_d[:])
        nc.sync.dma_start(ident_sb[:], ident_d[:])

        # ---- main loop: hT = W1.T @ xT, relu+bias, per-block sums.
        # The block-sum -> partial-ST pipeline (A-chunk generation on DVE,
        # PE transposes, ST accumulation matmuls) is interleaved: chunk k
        # is emitted as soon as its 128 blocks of bs_sb are reduced, so
        # the tail after the last xT DMA is only the final chunk + join.
        # segcol[p, k] = segment id of block k*128+p (-1 for pads)
        segcol = pp.tile([128, n_chunks], f32, name="segcol")
        nc.sync.dma_start(
            segcol[:], seg_d[0, :].rearrange("(c p) -> p c", p=128))
        iota_i = pp.tile([128, B], mybir.dt.int32, name="iota_i")
        iota_f = pp.tile([128, B], f32, name="iota_f")
        nc.gpsimd.iota(iota_i[:], pattern=[[1, B]], base=0,
                       channel_multiplier=0)
        nc.vector.tensor_copy(iota_f[:], iota_i[:])

        with (
            tc.tile_pool(name="xp", bufs=3) as xp,
            tc.tile_pool(name="rp", bufs=2) as rp,
            tc.tile_pool(name="hp", bufs=2, space="PSUM") as hp,
            tc.tile_pool(name="tp", bufs=2, space="PSUM") as tp,
            tc.tile_pool(name="ap", bufs=3) as ap_pool,
            tc.tile_pool(name="stp", bufs=1, space="PSUM") as stp,
        ):
            st_ps = stp.tile([128, B], f32, name="st_ps")

            def emit_chunk(k):
                jw = min(128, J - k * 128)
                tps = tp.tile([128, 128], f32, name="tps")
                nc.tensor.transpose(
                    tps[:jw, :], bs_sb[:, k * 128:k * 128 + jw], ident_sb[:])
                nc.scalar.copy(bsT_sb[:jw, k * 128:(k + 1) * 128], tps[:jw, :])
                at = ap_pool.tile([128, B], f32, name="at")
                nc.vector.tensor_scalar(
                    at[:], iota_f[:], segcol[:, k:k + 1], None,
                    op0=mybir.AluOpType.is_equal)
                for n0 in range(0, B, 512):
                    nw = min(512, B - n0)
                    nc.tensor.matmul(
                        st_ps[:, n0:n0 + nw],
                        lhsT=bsT_sb[:jw, k * 128:(k + 1) * 128],
                        rhs=at[:jw, n0:n0 + nw],
                        start=(k == 0), stop=(k == n_chunks - 1),
                    )

            next_chunk = 0
            for t in range(n_dma):
                xt = xp.tile([128, DMA_TILE], f32r, name="xt")
                nc.sync.dma_start(xt[:], xT_d[:, t * DMA_TILE:(t + 1) * DMA_TILE])
                for h in range(n_half):
                    ps = hp.tile([128, PSUM_TILE], f32, name="ps")
                    for q in range(PSUM_TILE // 512):
                        c0 = h * PSUM_TILE + q * 512
                        # float32r: fp32 data in PE replay mode — 1 cyc/row
                        # at n=512 vs 4 cyc/row for plain fp32
                        nc.tensor.matmul(
                            ps[:, q * 512:(q + 1) * 512],
                            lhsT=w1_sb[:],
                            rhs=xt[:, c0:c0 + 512],
                            start=True, stop=True,
                        )
                    rt = rp.tile([128, PSUM_TILE], f32, name="rt")
                    nc.scalar.activation(rt[:], ps[:], Relu, bias=b1_sb[:, 0:1])
                    j0 = (t * DMA_TILE + h * PSUM_TILE) // BLOCK
                    nc.vector.tensor_reduce(
                        bs_sb[:, j0:j0 + blk_per_ps],
                        rt[:].rearrange("p (j e) -> p j e", e=BLOCK),
                        axis=mybir.AxisListType.X,
                        op=mybir.AluOpType.add,
                    )
                    blocks_done = j0 + blk_per_ps
                    while (next_chunk < n_chunks
                           and blocks_done >= min(J, (next_chunk + 1) * 128)):
                        emit_chunk(next_chunk)
                        next_chunk += 1
            while next_chunk < n_chunks:
                emit_chunk(next_chunk)
                next_chunk += 1

            # every segment is wholly owned by one core (contiguous runs),
            # so the per-core partial ST is already the FINAL sum for this
            # core's segment window: no collective needed.  Each core runs
            # the (cheap) W2/W3 stage on its own window; the host stitches
            # the disjoint output row ranges together.
            sfull_sb = pp.tile([128, B], f32r, name="sfull_sb")
            nc.scalar.copy(sfull_sb[:], st_ps[:])

        # ---- final: sums_hT = W2.T @ S + b2 x counts; meanT; out ----
        w2_sb = pp.tile([128, 128], f32r, name="w2_sb")
        b2_sb = pp.tile([1, 128], f32r, name="b2_sb")
        w3_sb = pp.tile([128, 128], f32, name="w3_sb")
        b3_sb = pp.tile([1, 128], f32, name="b3_sb")
        cnt_sb = pp.tile([1, B], f32r, name="cnt_sb")
        inv_sb = pp.tile([128, B], f32, name="inv_sb")
        ones_sb = pp.tile([1, 128], f32, name="ones_sb")
        mean_sb = pp.tile([128, B], f32, name="mean_sb")
        nc.sync.dma_start(w2_sb[:], W2_d[:])
        nc.sync.dma_start(b2_sb[:], b2_d[:])
        nc.sync.dma_start(w3_sb[:], W3_d[:])
        nc.sync.dma_start(b3_sb[:], b3_d[:])
        nc.sync.dma_start(cnt_sb[:], cnt_d[:])
        nc.sync.dma_start(inv_sb[:], inv_d[:])
        nc.sync.dma_start(ones_sb[:], ones_d[:])

        with (
            tc.tile_pool(name="sp", bufs=1, space="PSUM") as sp,
            tc.tile_pool(name="op", bufs=2, space="PSUM") as op,
            tc.tile_pool(name="op_sb", bufs=8) as op_sb,
        ):
            sums_ps = sp.tile([128, B], f32, name="sums_ps")
            for n0 in range(0, B, 512):
                sl = slice(n0, min(n0 + 512, B))
                nc.tensor.matmul(sums_ps[:, sl], lhsT=w2_sb[:],
                                 rhs=sfull_sb[:, sl], start=True, stop=False)
                nc.tensor.matmul(sums_ps[:, sl], lhsT=b2_sb[0:1, :],
                                 rhs=cnt_sb[0:1, sl], start=False, stop=True)
            nc.vector.tensor_mul(mean_sb[:], sums_ps[:], inv_sb[:])

            # all B//128 output chunks into one psum tile (each matmul
            # writes its own 512B bank-aligned 128-col slice), one copy,
            # one rearranged DMA to the [B, 128] output
            n_oc = (B + 127) // 128
            ops = op.tile([128, n_oc * 128], f32, name="ops")
            for c in range(n_oc):
                c0 = c * 128
                cw = min(128, B - c0)
                nc.tensor.matmul(ops[:cw, c0:c0 + 128],
                                 lhsT=mean_sb[:, c0:c0 + cw],
                                 rhs=w3_sb[:], start=True, stop=False)
                nc.tensor.matmul(ops[:cw, c0:c0 + 128],
                                 lhsT=ones_sb[0:1, :cw],
                                 rhs=b3_sb[0:1, :], start=False, stop=True)
            osb = op_sb.tile([128, n_oc * 128], f32, name="osb")
            nc.scalar.copy(osb[:], ops[:])
            if B % 128 == 0 and n_oc > 1:
                nc.sync.dma_start(
                    out_d[:].rearrange("(c p) f -> p c f", p=128),
                    osb[:].rearrange("p (c f) -> p c f", f=128))
            else:
                for c in range(n_oc):
                    c0 = c * 128
                    cw = min(128, B - c0)
                    nc.sync.dma_start(out_d[c0:c0 + cw, :],
                                      osb[:cw, c0:c0 + 128])

    nc.compile()
    return nc


def _prepare(x1, edge_slices, W1, b1, W2, b2, W3, b3):
    """Host planning + per-core input construction + Bass program build."""
    x1 = np.ascontiguousarray(np.asarray(x1, dtype=np.float32))
    W1 = np.asarray(W1, dtype=np.float32)
    b1 = np.asarray(b1, dtype=np.float32)
    E = x1.shape[0]
    B = int(np.asarray(edge_slices).shape[0]) - 1

    es, counts, seg_blocks, bounds, e_cap = _plan_shards(edge_slices, E, B)
    x_pad = _solve_xpad(W1, b1)
    xTs, segids, J = _build_core_inputs(x1, es, counts, seg_blocks, bounds,
                                        e_cap, x_pad, B)

    counts_f = counts.astype(np.float32)
    inv = (1.0 / np.maximum(counts_f, 1.0)).astype(np.float32)
    shared = {
        "W1": W1,
        "b1c": np.ascontiguousarray(b1.reshape(128, 1)),
        "W2": np.asarray(W2, dtype=np.float32),
        "b2r": np.ascontiguousarray(np.asarray(b2, np.float32).reshape(1, 128)),
        "W3": np.asarray(W3, dtype=np.float32),
        "b3r": np.ascontiguousarray(np.asarray(b3, np.float32).reshape(1, 128)),
        "counts_row": np.ascontiguousarray(counts_f.reshape(1, B)),
        "inv_bcast": np.ascontiguousarray(np.repeat(inv.reshape(1, B), 128, axis=0)),
        "ones_row": np.ones((1, 128), np.float32),
        "ident": np.eye(128, dtype=np.float32),
    }

    nc = _build_bass(e_cap, J, B)
    in_maps = [{"xT": xTs[c], "segid": segids[c], **shared}
               for c in range(N_CORES)]
    return nc, in_maps, bounds


def _assemble(outs, bounds, B):
    out = np.empty((B, 128), dtype=np.float32)
    for c in range(N_CORES):
        out[bounds[c]:bounds[c + 1], :] = outs[c][bounds[c]:bounds[c + 1], :]
    return out


def kernel(x1, edge_slices, W1, b1, W2, b2, W3, b3):
    from concourse import bass_utils

    nc, in_maps, bounds = _prepare(x1, edge_slices, W1, b1, W2, b2, W3, b3)
    br = bass_utils.run_bass_kernel_spmd(
        nc, in_maps, core_ids=list(range(N_CORES))
    )
    B = int(np.asarray(edge_slices).shape[0]) - 1
    return _assemble([r["out"] for r in br.results], bounds, B)


# revision 22
# speedup vs baseline: 4.8227x; 4.8227x over previous
"""DeepSet segment-reduce kernel for 8 Trainium2 NeuronCores (Bass/Tile).

Math (reference):
    h  = relu(x1 @ W1 + b1) @ W2 + b2          # [E, 128]
    S  = segment_sum(h, seg)                   # [B, 128]
    mean = S / max(counts, 1)
    out  = mean @ W3 + b3

Because segment-sum is linear, only r = relu(x1 @ W1 + b1) needs per-edge
work:  segsum(h) = segsum(r) @ W2 + counts x b2.  So the kernel:

  host: repack edges so every segment starts on a 128-edge block boundary
        (pad columns use x_pad with W1.T x_pad + b1 < 0, so relu kills them
        exactly), shard segment runs across 8 cores balanced by block count.
  core: stream xT [128, E_cap] tiles; hT = W1.T @ xT (PE, W1 stationary);
        relu+bias on ACT (PSUM->SBUF); per-128-block sums on DVE (3-D
        tensor_reduce);  ST[f, b] = block_sumsT @ A  via PE transposes +
        matmuls where A [J, B] is the per-core one-hot block->segment map
        (an input tensor, so the instruction stream is identical on all
        cores -> one SPMD program);
        AllReduce partial ST over the 8 cores;
        sums_hT = W2.T @ S + b2 x counts (rank-1 via k=1 matmul);
        meanT   = sums_hT * inv_counts (broadcast tensor passed as input);
        out     = meanT.T @ W3 + 1 x b3 (rank-1), DMA to [B, 128] output.

Self-contained: no reads of /root/problem/*; shapes derived from inputs.
"""

import math

import numpy as np

N_CORES = 8
BLOCK = 128          # segment alignment quantum (edges per block)
DMA_TILE = 4096      # xT columns per DMA (2 MiB)
PSUM_TILE = 1024     # columns per PSUM tile / ACT op (2 banks)
PAD_MARGIN = 8.0


def _plan_shards(edge_slices, E, B):
    es = np.asarray(edge_slices, dtype=np.int64)
    counts = (es[1:] - es[:-1]).astype(np.int64)        # [B]
    seg_blocks = (counts + BLOCK - 1) // BLOCK          # [B]
    total_blocks = int(seg_blocks.sum())

    # contiguous runs of segments per core, balanced by block count
    cum = np.cumsum(seg_blocks)
    bounds = [0]
    for c in range(1, N_CORES):
        bounds.append(int(np.searchsorted(cum, c * total_blocks / N_CORES)))
    bounds.append(B)

    core_blocks = []
    for c in range(N_CORES):
        core_blocks.append(int(seg_blocks[bounds[c]:bounds[c + 1]].sum()))
    j_max = max(core_blocks)
    e_cap = int(math.ceil(j_max * BLOCK / DMA_TILE) * DMA_TILE)
    return es, counts, seg_blocks, bounds, e_cap


def _solve_xpad(W1, b1):
    # x_pad with W1.T x_pad + b1 = -PAD_MARGIN elementwise => relu output 0
    rhs = -(b1.astype(np.float64) + PAD_MARGIN)
    x_pad = np.linalg.solve(W1.astype(np.float64).T, rhs)
    chk = W1.astype(np.float64).T @ x_pad + b1.astype(np.float64)
    assert chk.max() < -1.0, "x_pad margin too small"
    return x_pad.astype(np.float32)


def _build_core_inputs(x1, es, counts, seg_blocks, bounds, e_cap, x_pad, B):
    J = e_cap // BLOCK
    J_pad = ((J + 127) // 128) * 128
    xTs, segids = [], []
    for c in range(N_CORES):
        xT = np.empty((128, e_cap), dtype=np.float32)
        segid = np.full((1, J_pad), -1.0, dtype=np.float32)
        pos = 0
        for b in range(bounds[c], bounds[c + 1]):
            cnt = int(counts[b])
            if cnt == 0:
                continue
            xT[:, pos:pos + cnt] = x1[es[b]:es[b + 1], :].T
            nb = int(seg_blocks[b])
            pad = nb * BLOCK - cnt
            if pad:
                xT[:, pos + cnt:pos + nb * BLOCK] = x_pad[:, None]
            segid[0, pos // BLOCK: pos // BLOCK + nb] = float(b)
            pos += nb * BLOCK
        if pos < e_cap:
            xT[:, pos:] = x_pad[:, None]
        xTs.append(xT)
        segids.append(segid)
    return xTs, segids, J


def _build_bass(e_cap, J, B, repeat=1):
    n_chunks = (J + 127) // 128
    J_pad = n_chunks * 128
    import concourse.bacc as bacc
    import concourse.mybir as mybir
    import concourse.tile as tile

    f32 = mybir.dt.float32
    Relu = mybir.ActivationFunctionType.Relu

    nc = bacc.Bacc(trn_type="TRN2", num_devices=N_CORES)

    f32r = mybir.dt.float32r
    xT_d = nc.dram_tensor("xT", [128, e_cap], f32r, kind="ExternalInput")
    seg_d = nc.dram_tensor("segid", [1, J_pad], f32, kind="ExternalInput")
    W1_d = nc.dram_tensor("W1", [128, 128], f32r, kind="ExternalInput")
    b1_d = nc.dram_tensor("b1c", [128, 1], f32, kind="ExternalInput")
    W2_d = nc.dram_tensor("W2", [128, 128], f32r, kind="ExternalInput")
    b2_d = nc.dram_tensor("b2r", [1, 128], f32r, kind="ExternalInput")
    W3_d = nc.dram_tensor("W3", [128, 128], f32, kind="ExternalInput")
    b3_d = nc.dram_tensor("b3r", [1, 128], f32, kind="ExternalInput")
    cnt_d = nc.dram_tensor("counts_row", [1, B], f32r, kind="ExternalInput")
    inv_d = nc.dram_tensor("inv_bcast", [128, B], f32, kind="ExternalInput")
    ones_d = nc.dram_tensor("ones_row", [1, 128], f32, kind="ExternalInput")
    ident_d = nc.dram_tensor("ident", [128, 128], f32, kind="ExternalInput")
    out_d = nc.dram_tensor("out", [B, 128], f32, kind="ExternalOutput")

    n_dma = e_cap // DMA_TILE
    n_half = DMA_TILE // PSUM_TILE
    blk_per_ps = PSUM_TILE // BLOCK

    with tile.TileContext(nc) as tc, tc.tile_pool(name="persist", bufs=1) as pp:
        prev_last = [None]

        def emit_body(rep):
          # one full pass of the kernel; rep > 0 only exists for the
          # repeat-timing harness (same work re-emitted, serialized on rep-1)
          sx = f"_{rep}" if rep else ""

          def dep_on_prev(inst):
              if prev_last[0] is not None:
                  from concourse.tile_rust import add_dep_helper
                  add_dep_helper(inst.ins, prev_last[0].ins, sync=True,
                                 reason="repeat-timing serialization")

          w1_sb = pp.tile([128, 128], f32r, name=f"w1_sb{sx}")
          b1_sb = pp.tile([128, 1], f32, name=f"b1_sb{sx}")
          ident_sb = pp.tile([128, 128], f32, name=f"ident_sb{sx}")
          bs_sb = pp.tile([128, J], f32, name=f"bs_sb{sx}")
          bsT_sb = pp.tile([128, n_chunks * 128], f32, name=f"bsT_sb{sx}")
          nc.sync.dma_start(w1_sb[:], W1_d[:])
          nc.sync.dma_start(b1_sb[:], b1_d[:])
          nc.sync.dma_start(ident_sb[:], ident_d[:])

          # segcol[p, k] = segment id of block k*128+p (-1 for pads)
          segcol = pp.tile([128, n_chunks], f32, name=f"segcol{sx}")
          nc.sync.dma_start(
              segcol[:], seg_d[0, :].rearrange("(c p) -> p c", p=128))
          iota_i = pp.tile([128, B], mybir.dt.int32, name=f"iota_i{sx}")
          iota_f = pp.tile([128, B], f32, name=f"iota_f{sx}")
          nc.gpsimd.iota(iota_i[:], pattern=[[1, B]], base=0,
                         channel_multiplier=0)
          nc.vector.tensor_copy(iota_f[:], iota_i[:])

          # main loop: hT = W1.T @ xT (f32r, 1 cyc/row), relu+bias on ACT,
          # per-128-block sums on DVE.  The block-sum -> partial-ST pipeline
          # (A-chunk gen on DVE via iota==segid, PE transposes, ST
          # accumulation matmuls) is interleaved so the post-DMA tail is
          # only the final chunk + the small W2/W3 stage.
          with (
              tc.tile_pool(name=f"xp{sx}", bufs=4) as xp,
              tc.tile_pool(name=f"rp{sx}", bufs=2) as rp,
              tc.tile_pool(name=f"hp{sx}", bufs=2, space="PSUM") as hp,
              tc.tile_pool(name=f"tp{sx}", bufs=2, space="PSUM") as tp,
              tc.tile_pool(name=f"ap{sx}", bufs=3) as ap_pool,
              tc.tile_pool(name=f"stp{sx}", bufs=1, space="PSUM") as stp,
          ):
            st_ps = stp.tile([128, B], f32, name=f"st_ps{sx}")

            def emit_chunk(k):
                jw = min(128, J - k * 128)
                tps = tp.tile([128, 128], f32, name=f"tps{sx}")
                nc.tensor.transpose(
                    tps[:jw, :], bs_sb[:, k * 128:k * 128 + jw], ident_sb[:])
                nc.scalar.copy(bsT_sb[:jw, k * 128:(k + 1) * 128], tps[:jw, :])
                at = ap_pool.tile([128, B], f32, name=f"at{sx}")
                nc.vector.tensor_scalar(
                    at[:], iota_f[:], segcol[:, k:k + 1], None,
                    op0=mybir.AluOpType.is_equal)
                for n0 in range(0, B, 512):
                    nw = min(512, B - n0)
                    nc.tensor.matmul(
                        st_ps[:, n0:n0 + nw],
                        lhsT=bsT_sb[:jw, k * 128:(k + 1) * 128],
                        rhs=at[:jw, n0:n0 + nw],
                        start=(k == 0), stop=(k == n_chunks - 1),
                    )

            next_chunk = 0
            for t in range(n_dma):
                xt = xp.tile([128, DMA_TILE], f32r, name=f"xt{sx}")
                if t == n_dma - 1:
                    # split the last tile so the dependent drain pipeline
                    # starts as early as possible
                    for s in range(0, DMA_TILE, PSUM_TILE):
                        di = nc.sync.dma_start(
                            xt[:, s:s + PSUM_TILE],
                            xT_d[:, t * DMA_TILE + s:t * DMA_TILE + s + PSUM_TILE])
                        dep_on_prev(di)
                else:
                    di = nc.sync.dma_start(
                        xt[:], xT_d[:, t * DMA_TILE:(t + 1) * DMA_TILE])
                    dep_on_prev(di)
                for h in range(n_half):
                    ps = hp.tile([128, PSUM_TILE], f32, name=f"ps{sx}")
                    for q in range(PSUM_TILE // 512):
                        c0 = h * PSUM_TILE + q * 512
                        nc.tensor.matmul(
                            ps[:, q * 512:(q + 1) * 512],
                            lhsT=w1_sb[:],
                            rhs=xt[:, c0:c0 + 512],
                            start=True, stop=True,
                        )
                    rt = rp.tile([128, PSUM_TILE], f32, name=f"rt{sx}")
                    nc.scalar.activation(rt[:], ps[:], Relu, bias=b1_sb[:, 0:1])
                    j0 = (t * DMA_TILE + h * PSUM_TILE) // BLOCK
                    nc.vector.tensor_reduce(
                        bs_sb[:, j0:j0 + blk_per_ps],
                        rt[:].rearrange("p (j e) -> p j e", e=BLOCK),
                        axis=mybir.AxisListType.X,
                        op=mybir.AluOpType.add,
                    )
                    blocks_done = j0 + blk_per_ps
                    while (next_chunk < n_chunks
                           and blocks_done >= min(J, (next_chunk + 1) * 128)):
                        emit_chunk(next_chunk)
                        next_chunk += 1
            while next_chunk < n_chunks:
                emit_chunk(next_chunk)
                next_chunk += 1

            # every segment is wholly owned by one core (contiguous runs),
            # so the per-core partial ST is already the FINAL sum for this
            # core's segment window: no collective needed.  Each core runs
            # the (cheap) W2/W3 stage on its own window; the host stitches
            # the disjoint output row ranges together.
            sfull_sb = pp.tile([128, B], f32r, name=f"sfull_sb{sx}")
            nc.scalar.copy(sfull_sb[:], st_ps[:])

          # final: sums_hT = W2.T @ S + b2 x counts; meanT; out
          w2_sb = pp.tile([128, 128], f32r, name=f"w2_sb{sx}")
          b2_sb = pp.tile([1, 128], f32r, name=f"b2_sb{sx}")
          w3_sb = pp.tile([128, 128], f32, name=f"w3_sb{sx}")
          b3_sb = pp.tile([1, 128], f32, name=f"b3_sb{sx}")
          cnt_sb = pp.tile([1, B], f32r, name=f"cnt_sb{sx}")
          inv_sb = pp.tile([128, B], f32, name=f"inv_sb{sx}")
          ones_sb = pp.tile([1, 128], f32, name=f"ones_sb{sx}")
          mean_sb = pp.tile([128, B], f32, name=f"mean_sb{sx}")
          nc.sync.dma_start(w2_sb[:], W2_d[:])
          nc.sync.dma_start(b2_sb[:], b2_d[:])
          nc.sync.dma_start(w3_sb[:], W3_d[:])
          nc.sync.dma_start(b3_sb[:], b3_d[:])
          nc.sync.dma_start(cnt_sb[:], cnt_d[:])
          nc.sync.dma_start(inv_sb[:], inv_d[:])
          nc.sync.dma_start(ones_sb[:], ones_d[:])

          with (
              tc.tile_pool(name=f"sp{sx}", bufs=1, space="PSUM") as sp,
              tc.tile_pool(name=f"op{sx}", bufs=2, space="PSUM") as op,
              tc.tile_pool(name=f"op_sb{sx}", bufs=1) as op_sb,
          ):
            sums_ps = sp.tile([128, B], f32, name=f"sums_ps{sx}")
            for n0 in range(0, B, 512):
                sl = slice(n0, min(n0 + 512, B))
                nc.tensor.matmul(sums_ps[:, sl], lhsT=w2_sb[:],
                                 rhs=sfull_sb[:, sl], start=True, stop=False)
                nc.tensor.matmul(sums_ps[:, sl], lhsT=b2_sb[0:1, :],
                                 rhs=cnt_sb[0:1, sl], start=False, stop=True)
            nc.vector.tensor_mul(mean_sb[:], sums_ps[:], inv_sb[:])

            # all B//128 output chunks into one psum tile (each matmul
            # writes its own 512B bank-aligned 128-col slice), one copy,
            # one rearranged DMA to the [B, 128] output
            n_oc = (B + 127) // 128
            ops = op.tile([128, n_oc * 128], f32, name=f"ops{sx}")
            for c in range(n_oc):
                c0 = c * 128
                cw = min(128, B - c0)
                nc.tensor.matmul(ops[:cw, c0:c0 + 128],
                                 lhsT=mean_sb[:, c0:c0 + cw],
                                 rhs=w3_sb[:], start=True, stop=False)
                nc.tensor.matmul(ops[:cw, c0:c0 + 128],
                                 lhsT=ones_sb[0:1, :cw],
                                 rhs=b3_sb[0:1, :], start=False, stop=True)
            osb = op_sb.tile([128, n_oc * 128], f32, name=f"osb{sx}")
            nc.scalar.copy(osb[:], ops[:])
            if B % 128 == 0 and n_oc > 1:
                last = nc.sync.dma_start(
                    out_d[:].rearrange("(c p) f -> p c f", p=128),
                    osb[:].rearrange("p (c f) -> p c f", f=128))
            else:
                for c in range(n_oc):
                    c0 = c * 128
                    cw = min(128, B - c0)
                    last = nc.sync.dma_start(out_d[c0:c0 + cw, :],
                                             osb[:cw, c0:c0 + 128])
            prev_last[0] = last

        for rep in range(repeat):
            emit_body(rep)

    nc.compile()
    return nc


def _prepare(x1, edge_slices, W1, b1, W2, b2, W3, b3):
    """Host planning + per-core input construction + Bass program build."""
    x1 = np.ascontiguousarray(np.asarray(x1, dtype=np.float32))
    W1 = np.asarray(W1, dtype=np.float32)
    b1 = np.asarray(b1, dtype=np.float32)
    E = x1.shape[0]
    B = int(np.asarray(edge_slices).shape[0]) - 1

    es, counts, seg_blocks, bounds, e_cap = _plan_shards(edge_slices, E, B)
    x_pad = _solve_xpad(W1, b1)
    xTs, segids, J = _build_core_inputs(x1, es, counts, seg_blocks, bounds,
                                        e_cap, x_pad, B)

    counts_f = counts.astype(np.float32)
    inv = (1.0 / np.maximum(counts_f, 1.0)).astype(np.float32)
    shared = {
        "W1": W1,
        "b1c": np.ascontiguousarray(b1.reshape(128, 1)),
        "W2": np.asarray(W2, dtype=np.float32),
        "b2r": np.ascontiguousarray(np.asarray(b2, np.float32).reshape(1, 128)),
        "W3": np.asarray(W3, dtype=np.float32),
        "b3r": np.ascontiguousarray(np.asarray(b3, np.float32).reshape(1, 128)),
        "counts_row": np.ascontiguousarray(counts_f.reshape(1, B)),
        "inv_bcast": np.ascontiguousarray(np.repeat(inv.reshape(1, B), 128, axis=0)),
        "ones_row": np.ones((1, 128), np.float32),
        "ident": np.eye(128, dtype=np.float32),
    }

    nc = _build_bass(e_cap, J, B)
    in_maps = [{"xT": xTs[c], "segid": segids[c], **shared}
               for c in range(N_CORES)]
    return nc, in_maps, bounds


def _assemble(outs, bounds, B):
    out = np.empty((B, 128), dtype=np.float32)
    for c in range(N_CORES):
        out[bounds[c]:bounds[c + 1], :] = outs[c][bounds[c]:bounds[c + 1], :]
    return out


def kernel(x1, edge_slices, W1, b1, W2, b2, W3, b3):
    from concourse import bass_utils

    nc, in_maps, bounds = _prepare(x1, edge_slices, W1, b1, W2, b2, W3, b3)
    br = bass_utils.run_bass_kernel_spmd(
        nc, in_maps, core_ids=list(range(N_CORES))
    )
    B = int(np.asarray(edge_slices).shape[0]) - 1
    return _assemble([r["out"] for r in br.results], bounds, B)
